# revision 20
# baseline (speedup 1.0000x reference)
"""Paged segmented attention (softcap, GQA, vLLM-style block tables) on 8 trn2 cores.

Sharding: data-parallel over sequences (8 seqs -> 8 cores). The host gathers each
sequence's KV blocks via its block table and lays them out exactly as the device
kernel wants them; the device does both QK^T orientations (transposed scores for
the PV matmul, natural scores for the per-row segment max), tanh/exp on ScalarE,
the masked max on VectorE (tensor_mask_reduce), PV + acc-transpose on TensorE.
"""

import numpy as np

# static problem config (mirrors the reference nn.Module)
S = 8            # sequences (= cores)
Q = 128          # query tokens per sequence
NQH = 32         # query heads
HKV = 8          # kv heads
G = 4            # query heads per kv head
D = 128          # head size
BLK = 16         # kv-cache block size
MB = 128         # blocks per sequence
NSEG = 4         # segments
SPAN = 512       # keys per segment (ceil(2048/(4*32))*32)
L = NSEG * SPAN  # 2048 keys per sequence
NCORES = 8

MASK_NEG = -30000.0

_prog_cache = {}


def _build_program(scale: float, softcap: float):
    from contextlib import ExitStack

    import concourse.bass as bass
    import concourse.bacc as bacc
    import concourse.mybir as mybir
    import concourse.tile as tile

    dt = mybir.dt
    f32 = dt.float32
    f32r = dt.float32r
    Alu = mybir.AluOpType
    Act = mybir.ActivationFunctionType

    sc_in = float(scale) / float(softcap)  # tanh input scale

    nc = bacc.Bacc("TRN2", target_bir_lowering=False, debug=False)

    # DRAM I/O (per core). Layouts (free dims flattened):
    #  qT : [D, (h, g, q)]                     128 x 4096
    #  K  : [seg][D, (h, k)]                   4 x 128 x 4096
    #  VT : [seg][k', (c, h, d)]  (k = c*128+k')  4 x 128 x 4096
    #  out: [seg][h][q, (g, d)]                4 x 8 x 128 x 512
    qT_d = nc.dram_tensor("qT", [128, HKV * G * Q], f32r, kind="ExternalInput")
    K_d = nc.dram_tensor("K", [NSEG, 128, HKV * SPAN], f32r, kind="ExternalInput")
    VT_d = nc.dram_tensor("VT", [NSEG, 128, 4 * HKV * D], f32r, kind="ExternalInput")
    tri_d = nc.dram_tensor("tri", [128, 512], f32, kind="ExternalInput")
    trin_d = nc.dram_tensor("trin", [128, 512], f32r, kind="ExternalInput")
    id_d = nc.dram_tensor("ident", [128, 128], f32r, kind="ExternalInput")
    out_d = nc.dram_tensor("out", [NSEG, HKV, Q, G * D], f32, kind="ExternalOutput")

    with tile.TileContext(nc) as tc, ExitStack() as ctx:
        kp = ctx.enter_context(tc.tile_pool(name="kp", bufs=2))
        vp = ctx.enter_context(tc.tile_pool(name="vp", bufs=2))
        qp = ctx.enter_context(tc.tile_pool(name="qp", bufs=1))
        cons = ctx.enter_context(tc.tile_pool(name="cons", bufs=1))
        tp = ctx.enter_context(tc.tile_pool(name="tp", bufs=3))
        pp = ctx.enter_context(tc.tile_pool(name="pp", bufs=4))
        acs = ctx.enter_context(tc.tile_pool(name="acs", bufs=4))
        osb = ctx.enter_context(tc.tile_pool(name="osb", bufs=6))
        mp = ctx.enter_context(tc.tile_pool(name="mp", bufs=12))
        psT = ctx.enter_context(tc.tile_pool(name="psT", bufs=1, space="PSUM"))
        psN = ctx.enter_context(tc.tile_pool(name="psN", bufs=1, space="PSUM"))
        psA = ctx.enter_context(tc.tile_pool(name="psA", bufs=1, space="PSUM"))
        psB = ctx.enter_context(tc.tile_pool(name="psB", bufs=1, space="PSUM"))

        # seg-0 K first so the first unit's matmuls can start ASAP
        k0_t = kp.tile([128, HKV * SPAN], f32r, tag="k", name="k0_t")
        nc.sync.dma_start(k0_t[:, :512], K_d[0, :, :512])
        qT_t = qp.tile([128, HKV * G * Q], f32r)
        nc.sync.dma_start(qT_t[:, 0:512], qT_d[:, 0:512])
        nc.sync.dma_start(k0_t[:, 512:2048], K_d[0, :, 512:2048])
        tri_t = cons.tile([128, 512], f32)
        nc.sync.dma_start(tri_t[:], tri_d[:])
        trin_t = cons.tile([128, 512], f32r)
        nc.sync.dma_start(trin_t[:], trin_d[:])
        id_t = cons.tile([128, 128], f32r)
        nc.sync.dma_start(id_t[:], id_d[:])
        for h in range(1, HKV):
            nc.sync.dma_start(
                qT_t[:, h * 512 : (h + 1) * 512], qT_d[:, h * 512 : (h + 1) * 512]
            )
        nc.sync.dma_start(k0_t[:, 2048:], K_d[0, :, 2048:])
        v0_t = vp.tile([128, 4 * HKV * D], f32r, tag="v", name="v0_t")
        nc.sync.dma_start(v0_t[:, :2048], VT_d[0, :, :2048])
        nc.sync.dma_start(v0_t[:, 2048:], VT_d[0, :, 2048:])

        for seg in range(NSEG):
            if seg == 0:
                k_t, v_t = k0_t, v0_t
            else:
                k_t = kp.tile([128, HKV * SPAN], f32r, tag="k")
                nc.sync.dma_start(k_t[:, :2048], K_d[seg, :, :2048])
                nc.sync.dma_start(k_t[:, 2048:], K_d[seg, :, 2048:])
                v_t = vp.tile([128, 4 * HKV * D], f32r, tag="v")
                nc.sync.dma_start(v_t[:, :2048], VT_d[seg, :, :2048])
                nc.sync.dma_start(v_t[:, 2048:], VT_d[seg, :, 2048:])

            for hp in range(HKV // 2):
                m_t = mp.tile([128, 2 * G], f32, tag="m")
                pus = {}
                for hi in range(2):
                    h = hp * 2 + hi
                    qslab = qT_t[:, h * 512 : (h + 1) * 512]

                    # transposed scores: psTt[k', (c, g, q)] = sum_d K[d,k]*q[d,(g,q)]
                    psTt = psT.tile([128, 2048], f32, tag="pT")
                    for c in range(4):
                        nc.tensor.matmul(
                            psTt[:, c * 512 : (c + 1) * 512],
                            k_t[:, h * 512 + c * 128 : h * 512 + (c + 1) * 128],
                            qslab,
                            start=True,
                            stop=True,
                        )

                    tT = tp.tile([128, 2048], f32, tag="t")
                    nc.scalar.activation(tT[:], psTt[:], Act.Tanh, scale=sc_in)
                    if seg == NSEG - 1:
                        nc.vector.tensor_add(
                            tT[:, 1536:2048], tT[:, 1536:2048], tri_t[:]
                        )
                    pu = pp.tile([128, 2048], f32r, tag="p")
                    nc.scalar.activation(pu[:], tT[:], Act.Exp, scale=float(softcap))
                    pus[hi] = pu

                    # natural scores (g pairs) -> masked row max
                    for gp in range(G // 2):
                        sN = psN.tile([128, 1024], f32, tag="sN")
                        for gi in range(2):
                            g = gp * 2 + gi
                            last = seg == NSEG - 1
                            nc.tensor.matmul(
                                sN[:, gi * 512 : (gi + 1) * 512],
                                qT_t[:, h * 512 + g * 128 : h * 512 + (g + 1) * 128],
                                k_t[:, h * 512 : (h + 1) * 512],
                                start=True,
                                stop=not last,
                            )
                            if last:
                                nc.tensor.matmul(
                                    sN[:, gi * 512 : (gi + 1) * 512],
                                    id_t[:],
                                    trin_t[:],
                                    start=False,
                                    stop=True,
                                )
                        nc.vector.tensor_reduce(
                            m_t[:, hi * G + gp * 2 : hi * G + gp * 2 + 2],
                            sN.rearrange("p (b f) -> p b f", b=2),
                            mybir.AxisListType.X,
                            Alu.max,
                        )

                # r[q] = exp(-softcap * tanh(sc_in * m)) for both h of the pair
                r1 = mp.tile([128, 2 * G], f32, tag="r1")
                nc.scalar.activation(r1[:], m_t[:], Act.Tanh, scale=sc_in)
                r_t = mp.tile([128, 2 * G], f32, tag="r")
                nc.scalar.activation(r_t[:], r1[:], Act.Exp, scale=-float(softcap))

                for hi in range(2):
                    h = hp * 2 + hi
                    pu = pus[hi]
                    # PV: accT[d, (g,q)] = sum_k VT[k,d] * pu[k,(g,q)]
                    accT = psA.tile([128, 512], f32, tag="accT")
                    for c in range(4):
                        nc.tensor.matmul(
                            accT[:],
                            v_t[:, (c * 8 + h) * 128 : (c * 8 + h + 1) * 128],
                            pu[:, c * 512 : (c + 1) * 512],
                            start=(c == 0),
                            stop=(c == 3),
                        )
                    a_sb = acs.tile([128, 512], f32r, tag="a")
                    nc.vector.tensor_copy(a_sb[:], accT[:].bitcast(f32r))

                    accN = psB.tile([128, 512], f32r, tag="accN")
                    for g in range(G):
                        nc.tensor.matmul(
                            accN[:, g * 128 : (g + 1) * 128],
                            a_sb[:, g * 128 : (g + 1) * 128],
                            id_t[:],
                            is_transpose=True,
                            skip_group_check=True,
                        )
                    o_sb = osb.tile([128, 512], f32, tag="o")
                    nc.vector.tensor_copy(o_sb[:], accN[:].bitcast(f32))
                    for g in range(G):
                        nc.gpsimd.tensor_scalar_mul(
                            o_sb[:, g * 128 : (g + 1) * 128],
                            o_sb[:, g * 128 : (g + 1) * 128],
                            r_t[:, hi * G + g : hi * G + g + 1],
                        )
                    nc.gpsimd.dma_start(out_d[seg, h], o_sb[:])
    nc.finalize()
    return nc


def _shard_inputs(query, key_cache, value_cache, block_tables, seq_lens):
    """Pure data-movement sharding: per-sequence KV gather + layout transforms."""
    f32 = np.float32
    in_maps = []
    qidx = np.arange(Q)
    tri = np.where(
        np.tile(qidx, G)[None, :] < np.arange(128)[:, None], MASK_NEG, 0.0
    ).astype(f32)
    ident = np.eye(128, dtype=f32)

    for s in range(S):
        bl = np.asarray(block_tables[s])
        # K: [128blk, h, d, b] -> [seg][d][(h, k=m*16+b)]
        kc = np.ascontiguousarray(key_cache[bl, :, :, :, 0])  # [128, 8, 128, 16]
        K_in = (
            kc.reshape(NSEG, 32, HKV, D, BLK)
            .transpose(0, 3, 2, 1, 4)
            .reshape(NSEG, D, HKV * SPAN)
            .astype(f32, copy=False)
        )
        # V: [seg][k'][(c, h, d)] with k = c*128 + k'
        vc = np.asarray(value_cache[bl]).reshape(NSEG, 32, HKV, D, BLK)
        VT_in = (
            vc.transpose(0, 1, 4, 2, 3)               # [seg, m, b, h, d]
            .reshape(NSEG, SPAN, HKV, D)              # [seg, k, h, d]
            .reshape(NSEG, 4, 128, HKV, D)            # [seg, c, k', h, d]
            .transpose(0, 2, 1, 3, 4)                 # [seg, k', c, h, d]
            .reshape(NSEG, 128, 4 * HKV * D)
            .astype(f32, copy=False)
        )
        qs = np.asarray(query[s * Q : (s + 1) * Q])   # [q, H, d]
        qT_in = (
            qs.reshape(Q, HKV, G, D)
            .transpose(3, 1, 2, 0)                    # [d, h, g, q]
            .reshape(D, HKV * G * Q)
            .astype(f32, copy=False)
        )
        # causal window (seg 3): key j valid iff j <= ctx + q - 3*SPAN
        ctx_len = int(seq_lens[s]) - Q
        thresh = ctx_len + qidx - (NSEG - 1) * SPAN  # [q]
        trin = np.where(
            np.arange(SPAN)[None, :] > thresh[:, None], MASK_NEG, 0.0
        ).astype(f32)
        in_maps.append(
            {
                "qT": np.ascontiguousarray(qT_in),
                "K": np.ascontiguousarray(K_in),
                "VT": np.ascontiguousarray(VT_in),
                "tri": tri,
                "trin": trin,
                "ident": ident,
            }
        )
    return in_maps


last_results = None  # BassKernelResults of the most recent kernel() call


def kernel(
    query,
    key_cache,
    value_cache,
    block_tables,
    seq_lens,
    query_start_len,
    scale,
    k_scale,
    v_scale,
    softcap,
):
    global last_results
    from concourse.bass_utils import run_bass_kernel_spmd
    import os

    query = np.asarray(query)
    key_cache = np.asarray(key_cache)
    value_cache = np.asarray(value_cache)
    block_tables = np.asarray(block_tables)
    seq_lens = np.asarray(seq_lens)

    key = (float(scale), float(softcap))
    if key not in _prog_cache:
        _prog_cache[key] = _build_program(float(scale), float(softcap))
    nc = _prog_cache[key]

    in_maps = _shard_inputs(query, key_cache, value_cache, block_tables, seq_lens)

    trace = bool(int(os.environ.get("KERNEL_TRACE", "0")))
    res = run_bass_kernel_spmd(nc, in_maps, core_ids=list(range(NCORES)), trace=trace)
    last_results = res

    out = np.empty((S * Q, NQH, NSEG, D), dtype=np.float32)
    for s in range(S):
        o = res.results[s]["out"]  # [seg, h, q, (g, d)]
        o = o.reshape(NSEG, HKV, Q, G, D).transpose(2, 1, 3, 0, 4)  # [q, h, g, seg, d]
        out[s * Q : (s + 1) * Q] = o.reshape(Q, NQH, NSEG, D)
    return out
